# revision 19
# baseline (speedup 1.0000x reference)
"""Trainium2 Bass kernel for causal multi-head attention (B=2, S=2048, E=1024, H=16).

Sharding: 8 cores = 2 batches x 4 head-groups (4 heads each).
Each core computes its batch's QKV for its 4 heads, causal attention, and a
partial output projection; host sums the 4 group partials per batch + b_out.

All big matmuls run in float32r (TF32-like, 1 cycle/row at N>=256).
Perf structure:
  - inputs/weights DMA'd directly as float32r (no staging/round copies);
    host pre-shuffles layouts so each partition's DMA payload is contiguous.
  - startup DMAs split across both HWDGE rings (x on sync, weights on
    scalar) so the first QKV matmul starts ~6us in.
  - K=64 score matmul pairs run concurrently in the PE array (auto row
    tiling from base partitions 0/64).
  - next-chunk QKV + prev-chunk out-proj woven into the attention loop to
    keep the PE stream dense (HAM clock stays warm).
  - rollout drains po PSUM banks to SBUF immediately (freeing next chunk's
    PV accumulation), then normalizes off SBUF; tiny 4-row reciprocal and
    two cross-ring broadcast DMAs keep the chunk-boundary latency chain
    short.
"""
import sys

sys.path.insert(0, "/opt/trn_rl_repo")

from contextlib import ExitStack

import numpy as np

import concourse.bass as bass
import concourse.tile as tile
from concourse import bacc, mybir
from concourse.bass_utils import run_bass_kernel_spmd

dt = mybir.dt

B, S, E, H = 2, 2048, 1024, 16
HD = 64                     # head dim
HPC = 4                     # heads per core
NC = 8                      # cores
KE = E // 128               # 8 contraction k-tiles for projections
NT = S // 128               # 16 token tiles
NCH = S // 512              # 4 token chunks
FQK = 512                   # q+k features per core (4 heads * 64 * 2)
FV = 256                    # v features per core
PDEPTH = 3                  # deferred-PV pipeline depth


def _build_program():
    nc = bacc.Bacc("TRN2", target_bir_lowering=False, debug=False, num_devices=NC)

    x_d = nc.dram_tensor("x", [128, NCH, KE, 512], dt.float32r, kind="ExternalInput")
    wqk_d = nc.dram_tensor("wqk", [128, 4, KE, 128], dt.float32r, kind="ExternalInput")
    wv_d = nc.dram_tensor("wv", [128, KE, FV], dt.float32r, kind="ExternalInput")
    wo_d = nc.dram_tensor("wo", [128, 2, E], dt.float32r, kind="ExternalInput")
    bqk_d = nc.dram_tensor("bqk", [128, 4], dt.float32, kind="ExternalInput")
    bv_d = nc.dram_tensor("bv", [128, 2], dt.float32, kind="ExternalInput")
    mask_d = nc.dram_tensor("trimask", [128, 128], dt.float32, kind="ExternalInput")
    y_d = nc.dram_tensor("y", [S, E], dt.float32, kind="ExternalOutput")

    with TileKernel(nc) as tk:
        tk.build(x_d, wqk_d, wv_d, wo_d, bqk_d, bv_d, mask_d, y_d)
    nc.compile()
    return nc


class TileKernel:
    def __init__(self, nc):
        self.nc = nc
        self.ctx = ExitStack()
        self.tc_cm = tile.TileContext(nc)

    def __enter__(self):
        self.tc = self.tc_cm.__enter__()
        return self

    def __exit__(self, *a):
        self.ctx.close()
        return self.tc_cm.__exit__(*a)

    def build(self, x_d, wqk_d, wv_d, wo_d, bqk_d, bv_d, mask_d, y_d):
        nc, tc, ctx = self.nc, self.tc, self.ctx
        pool = lambda name, bufs, **kw: ctx.enter_context(
            tc.tile_pool(name=name, bufs=bufs, **kw)
        )

        const_p = pool("const", 1)
        xr_p = pool("xr", 2)
        qkt_p = pool("qkt", 1)
        vones_p = pool("vones", 1)
        attn_p = pool("attn", PDEPTH + 1)
        pair_p = pool("pair", 1)
        small_p = pool("small", 1)
        y_p = pool("y", 4)
        # PSUM: ps (2 banks x 2 bufs) + po (1 bank x 1 buf x 4 tags) = 8
        ps_p = pool("ps", 2, space="PSUM")
        po_p = pool("po", 1, space="PSUM")

        # ---- startup DMAs: x chunk 0 streams per-ke on the sync HWDGE
        # ring while wqk streams per-f on the scalar ring, so the first
        # QKV matmuls chase the DMA stream instead of waiting for 4MB
        xr0 = xr_p.tile([128, KE, 512], dt.float32r, tag="xr", name="xr0")
        wqk_sb = const_p.tile([128, 4, KE, 128], dt.float32r, tag="wqk")
        nc.sync.dma_start(xr0[:, 0], x_d[:, 0, 0])
        nc.scalar.dma_start(wqk_sb[:, 0], wqk_d[:, 0])
        for ke in range(1, KE):
            nc.sync.dma_start(xr0[:, ke], x_d[:, 0, ke])
        for f in range(1, 4):
            nc.scalar.dma_start(wqk_sb[:, f], wqk_d[:, f])
        bqk_sb = const_p.tile([128, 4], dt.float32, tag="bqk")
        nc.sync.dma_start(bqk_sb[:], bqk_d[:])
        bv_sb = const_p.tile([128, 2], dt.float32, tag="bv")
        nc.sync.dma_start(bv_sb[:], bv_d[:])
        mask_sb = const_p.tile([128, 128], dt.float32, tag="mask")
        nc.sync.dma_start(mask_sb[:], mask_d[:])
        xr1 = xr_p.tile([128, KE, 512], dt.float32r, tag="xr", name="xr1")
        nc.sync.dma_start(xr1[:], x_d[:, 1])
        wv_sb = const_p.tile([128, KE, FV], dt.float32r, tag="wv")
        nc.scalar.dma_start(wv_sb[:], wv_d[:])
        wo_sb = const_p.tile([128, 2, E], dt.float32r, tag="wo")
        nc.scalar.dma_start(wo_sb[:], wo_d[:])
        wo_r = [wo_sb[:, kt] for kt in range(2)]
        ones_sb = const_p.tile([128, 1, 1], dt.float32, tag="ones")
        nc.vector.memset(ones_sb[:], 1.0)
        # selector matrix for the 1/den broadcast matmuls: within a K=33
        # slice based at partition 64*hp, row 0 carries head 2hp (output
        # partitions 0:64) and row 32 carries head 2hp+1 (partitions 64:128)
        sel_f = const_p.tile([128, 128], dt.float32, tag="sel_f")
        nc.vector.memset(sel_f[:], 0.0)
        for rp, mlo in ((0, 0), (32, 64), (64, 0), (96, 64)):
            nc.vector.memset(sel_f[rp : rp + 1, mlo : mlo + 64], 1.0)
        sel = const_p.tile([128, 128], dt.float32r, tag="sel")
        nc.vector.tensor_copy(sel[:], sel_f[:])
        # 1/den staging rows at partitions 0/32/64/96 (one per head)
        recip4 = const_p.tile([128, 512], dt.float32, tag="recip4")
        nc.vector.memset(recip4[:], 1.0)
        recip4r = const_p.tile([128, 512], dt.float32r, tag="recip4r")
        nc.vector.tensor_copy(recip4r[:], recip4[:])

        # ---- persistent activations ----
        # qkt tiles: 0: q heads 0,1 | 1: q heads 2,3 | 2: k heads 0,1 | 3: k heads 2,3
        qkt = [qkt_p.tile([128, S], dt.float32r, tag=f"qkt{f}", name=f"qkt{f}") for f in range(4)]
        # vones[t]: [v h0 |1| v h1 |1| v h2 |1| v h3 |1] for token tile t
        vones = [vones_p.tile([128, 4 * 65], dt.float32r, tag=f"v{t}", name=f"v{t}") for t in range(NT)]
        # pair tiles: final normalized attn output, [head dims x 2, S]
        pairt = [pair_p.tile([128, S], dt.float32r, tag=f"pair{hp}", name=f"pair{hp}") for hp in range(2)]

        env = dict(
            x_d=x_d, wqk_sb=wqk_sb, wv_sb=wv_sb, bqk_sb=bqk_sb, ones_sb=ones_sb,
            xr_p=xr_p, qkt=qkt, vones=vones, xr_pre={0: xr0, 1: xr1},
            ps_p=ps_p, po_p=po_p, attn_p=attn_p, small_p=small_p,
            pairt=pairt, bv_sb=bv_sb, mask_sb=mask_sb, wo_r=wo_r,
            y_p=y_p, y_d=y_d, recip4=recip4, recip4r=recip4r, sel=sel,
        )
        # startup: chunk-0 qkv emitted directly
        for u in self.qkv_units(0, env):
            u()
        # filler composition per chunk: a few qkv units first (so the
        # deferred rollout's recip is ready before its bch matmul), then
        # rollout phase-2, then the rest.  oproj(1) is deferred from chunk 2
        # to chunk 3 to balance PE work against chunk 3's exp-heavy scalar
        # load.
        pending_rollout = []
        oproj_sched = {1: [0], 2: [], 3: [1, 2]}
        for c in range(NCH):
            qkv = self.qkv_units(c + 1, env) if c + 1 < NCH else []
            op = []
            for oc in oproj_sched.get(c, []):
                op += self.oproj_units(oc, env)
            if qkv:
                fillers = qkv[0:4] + pending_rollout + qkv[4:] + op
            else:
                fillers = op[0:2] + pending_rollout + op[2:]
            pending_rollout = self.attention_chunk(c, env, fillers)
        for u in pending_rollout:
            u()
        for u in self.oproj_units(NCH - 1, env, final=True):
            u()

    # ------------------------------------------------------------------
    def qkv_units(self, c, env):
        nc = self.nc
        cs = slice(512 * c, 512 * (c + 1))
        x_d, wqk_sb, wv_sb = env["x_d"], env["wqk_sb"], env["wv_sb"]
        qkt, vones = env["qkt"], env["vones"]
        bqk_sb, ones_sb = env["bqk_sb"], env["ones_sb"]
        xr_p, ps_p = env["xr_p"], env["ps_p"]

        if c in env["xr_pre"]:
            xr = env["xr_pre"][c]
            units = []
        else:
            xr = xr_p.tile([128, KE, 512], dt.float32r, tag="xr", name="xr")
            units = [lambda: nc.gpsimd.dma_start(xr[:], x_d[:, c])]

        def qk_unit(f):
            pq = ps_p.tile([128, 1024], dt.float32, tag="ps", name="pq")
            for ke in range(KE):
                nc.tensor.matmul(
                    pq[:, 0:512], wqk_sb[:, f, ke], xr[:, ke],
                    start=(ke == 0), stop=(ke == KE - 1),
                )
            nc.vector.tensor_scalar_add(qkt[f][:, cs], pq[:, 0:512], bqk_sb[:, f : f + 1])

        def v_unit(t4):
            t = 4 * c + t4
            pv = ps_p.tile([128, 1024], dt.float32, tag="ps", name="pv")
            for ke in range(KE):
                nc.tensor.matmul(
                    pv[:, 0:FV],
                    xr[:, ke, 128 * t4 : 128 * (t4 + 1)], wv_sb[:, ke],
                    start=(ke == 0), stop=(ke == KE - 1),
                )
            vt = vones[t]
            v3 = vt[:].rearrange("p (g d) -> p g d", d=65)
            nc.vector.tensor_copy(
                v3[:, :, 0:64],
                pv[:, 0:FV].rearrange("p (g d) -> p g d", d=64),
            )
            nc.vector.tensor_copy(v3[:, :, 64:65], ones_sb[:].to_broadcast((128, 4, 1)))

        for f in range(4):
            units.append(lambda f=f: qk_unit(f))
        for t4 in range(4):
            units.append(lambda t4=t4: v_unit(t4))
        return units

    # ------------------------------------------------------------------
    def oproj_units(self, c, env, final=False):
        nc = self.nc
        pairt, wo_r, ps_p, y_p, y_d = (
            env["pairt"], env["wo_r"], env["ps_p"], env["y_p"], env["y_d"])
        units = []
        ysbs = {}

        def unit(t4, o):
            t = 4 * c + t4
            if o == 0:
                ysbs[t4] = y_p.tile([128, E], dt.float32, tag="y", name="ysb")
            ysb = ysbs[t4]
            py = ps_p.tile([128, 1024], dt.float32, tag="ps", name="py")
            for kt in range(2):
                nc.tensor.matmul(
                    py[:, 0:512],
                    pairt[kt][:, 128 * t : 128 * (t + 1)],
                    wo_r[kt][:, 512 * o : 512 * (o + 1)],
                    start=(kt == 0), stop=(kt == 1),
                )
            nc.vector.tensor_copy(ysb[:, 512 * o : 512 * (o + 1)], py[:, 0:512])
            if o == 1:
                if final:
                    eng = (nc.sync, nc.gpsimd, nc.scalar, nc.gpsimd)[t % 4]
                else:
                    eng = (nc.sync, nc.gpsimd)[t % 2]
                eng.dma_start(y_d[128 * t : 128 * (t + 1), :], ysb[:])

        for t4 in range(4):
            for o in range(2):
                units.append(lambda t4=t4, o=o: unit(t4, o))
        return units

    # ------------------------------------------------------------------
    def attention_chunk(self, c, env, fillers):
        """Attention for both head pairs of chunk c, weaving filler units
        (next-chunk qkv / prev-chunk out-proj) into the PE stream."""
        nc = self.nc
        qkt, vones = env["qkt"], env["vones"]
        ps_p, po_p, attn_p, small_p = (
            env["ps_p"], env["po_p"], env["attn_p"], env["small_p"])
        pairt, bv_sb, mask_sb = env["pairt"], env["bv_sb"], env["mask_sb"]
        nj = 4 * c + 4
        # po[2*hp + h_idx]: [65, 512] accumulator per head
        po = [po_p.tile([65, 512], dt.float32, tag=f"po{i}", name=f"po{i}")
              for i in range(4)]

        nreserve = min(2, len(fillers))
        nfill = len(fillers) - nreserve
        iters = 2 * nj
        emitted = 0

        def emit_pv(hp, j, off, at):
            for h_idx in range(2):
                slot = 2 * hp + h_idx
                nc.tensor.matmul(
                    po[slot][:, off:512],
                    vones[j][:, 65 * slot : 65 * slot + 65],
                    at[:, 512 * h_idx + off : 512 * (h_idx + 1)],
                    start=(j == 0), stop=(j == nj - 1),
                    skip_group_check=True,
                )

        it = 0
        for hp in range(2):
            pending = []
            for j in range(nj):
                ps = ps_p.tile([128, 1024], dt.float32, tag="ps", name="ps")
                at = attn_p.tile([128, 1024], dt.float32r, tag="attn", name="at")
                m = j - 4 * c
                off = 128 * m if 1 <= m <= 3 else 0
                for h_idx in range(2):
                    r0 = 64 * h_idx
                    nc.tensor.matmul(
                        ps[:, 512 * h_idx + off : 512 * (h_idx + 1)],
                        qkt[2 + hp][r0 : r0 + 64, 128 * j : 128 * (j + 1)],
                        qkt[hp][r0 : r0 + 64, 512 * c + off : 512 * (c + 1)],
                        start=True, stop=True,
                    )
                if m >= 0:
                    lo = 128 * m
                    nc.vector.tensor_add(
                        ps[:].rearrange("p (g n) -> p g n", g=2)[:, :, lo : lo + 128],
                        ps[:].rearrange("p (g n) -> p g n", g=2)[:, :, lo : lo + 128],
                        mask_sb[:].rearrange("p (o n) -> p o n", o=1)
                        .to_broadcast((128, 2, 128)))
                if off == 0:
                    runs = [(0, 1024)]
                else:
                    runs = [(off, 512), (512 + off, 1024)]
                for lo, hi in runs:
                    nc.scalar.activation(
                        at[:, lo:hi], ps[:, lo:hi], mybir.ActivationFunctionType.Exp)
                pending.append((j, off, at))
                if len(pending) > PDEPTH:
                    emit_pv(hp, *pending.pop(0))
                it += 1
                while emitted < nfill and emitted * iters < it * nfill:
                    fillers[emitted]()
                    emitted += 1
            for p in pending:
                emit_pv(hp, *p)
        while emitted < nfill:
            fillers[emitted]()
            emitted += 1
        # reserved fillers keep the PE stream dense across the chunk
        # boundary (HAM re-throttles if PE density dips for ~3.4us)
        for u in fillers[nfill:]:
            u()
        # ---- rollout: drain po to SBUF fast (frees PSUM for next chunk),
        # then normalize off SBUF while the next chunk proceeds ----
        recip4 = env["recip4"]
        for i in range(4):
            nc.vector.tensor_copy(recip4[32 * i : 32 * i + 1, :], po[i][64:65, :])
        po_sb = [
            small_p.tile([128, 512], dt.float32, tag=f"posb{hp}", name=f"posb{hp}")
            for hp in range(2)
        ]
        for hp in range(2):
            nc.vector.tensor_copy(po_sb[hp][0:64, :], po[2 * hp][0:64, :])
            nc.vector.tensor_copy(po_sb[hp][64:128, :], po[2 * hp + 1][0:64, :])
        # phase 2 (normalize off SBUF) is returned as filler units for the
        # next chunk so the PE stream never pauses at the chunk boundary
        recip4r = env["recip4r"]
        sel = env["sel"]
        # cheap approx reciprocal (full reciprocal has ~3.5us fixed cost);
        # vector-only, so it runs during the next chunk's first iterations
        nc.vector.reciprocal_approx_fast(recip4[:], recip4[:])
        nc.vector.tensor_copy(recip4r[:], recip4[:])

        bch_ps_box = []

        def u_bch():
            # broadcast 1/den across partitions with one selector matmul per
            # head pair (avoids slow 64-descriptor broadcast DMAs)
            bch_ps = ps_p.tile([128, 1024], dt.float32, tag="ps", name="bch")
            bch_ps_box.append(bch_ps)
            for hp in range(2):
                nc.tensor.matmul(
                    bch_ps[:, 512 * hp : 512 * (hp + 1)],
                    sel[64 * hp : 64 * hp + 33, :],
                    recip4r[64 * hp : 64 * hp + 33, :],
                    start=True, stop=True, skip_group_check=True,
                )

        def u_norm(hp):
            bch_ps = bch_ps_box[0]
            tmp = small_p.tile([128, 512], dt.float32, tag=f"tmp{hp}", name=f"tmp{hp}")
            nc.vector.tensor_mul(
                tmp[:], po_sb[hp][:], bch_ps[:, 512 * hp : 512 * (hp + 1)])
            nc.vector.tensor_scalar_add(
                pairt[hp][:, 512 * c : 512 * (c + 1)], tmp[:], bv_sb[:, hp : hp + 1]
            )

        return [u_bch] + [lambda hp=hp: u_norm(hp) for hp in range(2)]

# ----------------------------------------------------------------------
_PROGRAM = None


def _get_program():
    global _PROGRAM
    if _PROGRAM is None:
        _PROGRAM = _build_program()
    return _PROGRAM


def _make_in_maps(inputs, W_in, b_in, W_out, b_out):
    in_maps = []
    scale = 1.0 / np.sqrt(np.float32(HD))
    kr = np.arange(128)[:, None]
    qc = np.arange(128)[None, :]
    trimask = np.where(qc >= kr, 0.0, -1e30).astype(np.float32)
    for core in range(NC):
        b, g = divmod(core, 4)
        r = slice(256 * g, 256 * (g + 1))
        wq = W_in[0:E][r] * scale
        wk = W_in[E : 2 * E][r]
        wv = W_in[2 * E : 3 * E][r]
        xT = inputs[b].T.astype(np.float32)               # [E, S]
        wqkT = np.concatenate([wq, wk], axis=0).T         # [E, FQK]
        wvT = wv.T                                        # [E, FV]
        wo = W_out[:, r].T                                # [FV, E]
        # shuffle so each partition's DMA payload is contiguous
        x_h = np.ascontiguousarray(
            xT.reshape(KE, 128, NCH, 512).transpose(1, 2, 0, 3))
        wqk_h = np.ascontiguousarray(
            wqkT.reshape(KE, 128, 4, 128).transpose(1, 2, 0, 3).astype(np.float32))
        wv_h = np.ascontiguousarray(
            wvT.reshape(KE, 128, FV).transpose(1, 0, 2).astype(np.float32))
        wo_h = np.ascontiguousarray(
            wo.reshape(2, 128, E).transpose(1, 0, 2).astype(np.float32))
        bqk = np.concatenate([b_in[0:E][r] * scale, b_in[E : 2 * E][r]])
        bqk_h = np.ascontiguousarray(bqk.reshape(4, 128).T.astype(np.float32))
        bv_h = np.ascontiguousarray(
            b_in[2 * E : 3 * E][r].reshape(2, 128).T.astype(np.float32))
        in_maps.append(
            {
                "x": x_h,
                "wqk": wqk_h,
                "wv": wv_h,
                "wo": wo_h,
                "bqk": bqk_h,
                "bv": bv_h,
                "trimask": trimask,
            }
        )
    return in_maps


def run_spmd(inputs, W_in, b_in, W_out, b_out, trace=False, **kw):
    nc = _get_program()
    in_maps = _make_in_maps(inputs, W_in, b_in, W_out, b_out)
    bkr = run_bass_kernel_spmd(nc, in_maps, list(range(NC)), trace=trace, **kw)
    parts = [bkr.results[i]["y"] for i in range(NC)]
    out = np.stack(
        [
            parts[0] + parts[1] + parts[2] + parts[3],
            parts[4] + parts[5] + parts[6] + parts[7],
        ]
    )
    out = out + b_out[None, None, :]
    return out.astype(np.float32), bkr


def kernel(inputs, W_in, b_in, W_out, b_out):
    out, _ = run_spmd(
        np.asarray(inputs, dtype=np.float32),
        np.asarray(W_in, dtype=np.float32),
        np.asarray(b_in, dtype=np.float32),
        np.asarray(W_out, dtype=np.float32),
        np.asarray(b_out, dtype=np.float32),
    )
    return out


if __name__ == "__main__":
    import reference
    ins = {k: np.asarray(v) for k, v in reference.setup_inputs().items()}
    exp = np.asarray(reference.reference(**ins))
    got = kernel(**ins)
    rel = np.abs(got - exp).max() / np.abs(exp).max()
    print("rel", rel)


# revision 20
# speedup vs baseline: 1.0181x; 1.0181x over previous
"""Trainium2 Bass kernel for causal multi-head attention (B=2, S=2048, E=1024, H=16).

Sharding: 8 cores = 2 batches x 4 head-groups (4 heads each).
Each core computes its batch's QKV for its 4 heads, causal attention, and a
partial output projection; host sums the 4 group partials per batch + b_out.

All big matmuls run in float32r (TF32-like, 1 cycle/row at N>=256).
Perf structure:
  - inputs/weights DMA'd directly as float32r (no staging/round copies);
    host pre-shuffles layouts so each partition's DMA payload is contiguous.
  - startup DMAs split across both HWDGE rings (x on sync, weights on
    scalar) so the first QKV matmul starts ~6us in.
  - K=64 score matmul pairs run concurrently in the PE array (auto row
    tiling from base partitions 0/64).
  - next-chunk QKV + prev-chunk out-proj woven into the attention loop to
    keep the PE stream dense (HAM clock stays warm).
  - rollout drains po PSUM banks to SBUF immediately (freeing next chunk's
    PV accumulation), then normalizes off SBUF; tiny 4-row reciprocal and
    two cross-ring broadcast DMAs keep the chunk-boundary latency chain
    short.
"""
import sys

sys.path.insert(0, "/opt/trn_rl_repo")

from contextlib import ExitStack

import numpy as np

import concourse.bass as bass
import concourse.tile as tile
from concourse import bacc, mybir
from concourse.bass_utils import run_bass_kernel_spmd

dt = mybir.dt

B, S, E, H = 2, 2048, 1024, 16
HD = 64                     # head dim
HPC = 4                     # heads per core
NC = 8                      # cores
KE = E // 128               # 8 contraction k-tiles for projections
NT = S // 128               # 16 token tiles
NCH = S // 512              # 4 token chunks
FQK = 512                   # q+k features per core (4 heads * 64 * 2)
FV = 256                    # v features per core
PDEPTH = 3                  # deferred-PV pipeline depth


def _build_program():
    nc = bacc.Bacc("TRN2", target_bir_lowering=False, debug=False, num_devices=NC)

    x_d = nc.dram_tensor("x", [128, NCH, KE, 512], dt.float32r, kind="ExternalInput")
    wqk_d = nc.dram_tensor("wqk", [128, 4, KE, 128], dt.float32r, kind="ExternalInput")
    wv_d = nc.dram_tensor("wv", [128, KE, FV], dt.float32r, kind="ExternalInput")
    wo_d = nc.dram_tensor("wo", [128, 2, E], dt.float32r, kind="ExternalInput")
    bqk_d = nc.dram_tensor("bqk", [128, 4], dt.float32, kind="ExternalInput")
    bv_d = nc.dram_tensor("bv", [128, 2], dt.float32, kind="ExternalInput")
    mask_d = nc.dram_tensor("trimask", [128, 128], dt.float32, kind="ExternalInput")
    y_d = nc.dram_tensor("y", [S, E], dt.float32, kind="ExternalOutput")

    with TileKernel(nc) as tk:
        tk.build(x_d, wqk_d, wv_d, wo_d, bqk_d, bv_d, mask_d, y_d)
    nc.compile()
    return nc


class TileKernel:
    def __init__(self, nc):
        self.nc = nc
        self.ctx = ExitStack()
        self.tc_cm = tile.TileContext(nc)

    def __enter__(self):
        self.tc = self.tc_cm.__enter__()
        return self

    def __exit__(self, *a):
        self.ctx.close()
        return self.tc_cm.__exit__(*a)

    def build(self, x_d, wqk_d, wv_d, wo_d, bqk_d, bv_d, mask_d, y_d):
        nc, tc, ctx = self.nc, self.tc, self.ctx
        pool = lambda name, bufs, **kw: ctx.enter_context(
            tc.tile_pool(name=name, bufs=bufs, **kw)
        )

        const_p = pool("const", 1)
        xr_p = pool("xr", 2)
        qkt_p = pool("qkt", 1)
        vones_p = pool("vones", 1)
        attn_p = pool("attn", PDEPTH + 1)
        pair_p = pool("pair", 1)
        small_p = pool("small", 1)
        y_p = pool("y", 4)
        # PSUM: ps (2 banks x 2 bufs) + po (1 bank x 1 buf x 4 tags) = 8
        ps_p = pool("ps", 2, space="PSUM")
        po_p = pool("po", 1, space="PSUM")

        # ---- startup DMAs: x chunk 0 streams per-ke on the sync HWDGE
        # ring while wqk streams per-f on the scalar ring, so the first
        # QKV matmuls chase the DMA stream instead of waiting for 4MB
        xr0 = xr_p.tile([128, KE, 512], dt.float32r, tag="xr", name="xr0")
        wqk_sb = const_p.tile([128, 4, KE, 128], dt.float32r, tag="wqk")
        nc.sync.dma_start(xr0[:, 0], x_d[:, 0, 0])
        nc.scalar.dma_start(wqk_sb[:, 0], wqk_d[:, 0])
        for ke in range(1, KE):
            nc.sync.dma_start(xr0[:, ke], x_d[:, 0, ke])
        for f in range(1, 4):
            nc.scalar.dma_start(wqk_sb[:, f], wqk_d[:, f])
        bqk_sb = const_p.tile([128, 4], dt.float32, tag="bqk")
        nc.sync.dma_start(bqk_sb[:], bqk_d[:])
        bv_sb = const_p.tile([128, 2], dt.float32, tag="bv")
        nc.sync.dma_start(bv_sb[:], bv_d[:])
        mask_sb = const_p.tile([128, 128], dt.float32, tag="mask")
        nc.sync.dma_start(mask_sb[:], mask_d[:])
        xr1 = xr_p.tile([128, KE, 512], dt.float32r, tag="xr", name="xr1")
        nc.sync.dma_start(xr1[:], x_d[:, 1])
        wv_sb = const_p.tile([128, KE, FV], dt.float32r, tag="wv")
        nc.scalar.dma_start(wv_sb[:], wv_d[:])
        wo_sb = const_p.tile([128, 2, E], dt.float32r, tag="wo")
        nc.scalar.dma_start(wo_sb[:], wo_d[:])
        wo_r = [wo_sb[:, kt] for kt in range(2)]
        ones_sb = const_p.tile([128, 1, 1], dt.float32, tag="ones")
        nc.vector.memset(ones_sb[:], 1.0)
        # selector matrix for the 1/den broadcast matmuls: within a K=33
        # slice based at partition 64*hp, row 0 carries head 2hp (output
        # partitions 0:64) and row 32 carries head 2hp+1 (partitions 64:128)
        sel_f = const_p.tile([128, 128], dt.float32, tag="sel_f")
        nc.vector.memset(sel_f[:], 0.0)
        for rp, mlo in ((0, 0), (32, 64), (64, 0), (96, 64)):
            nc.vector.memset(sel_f[rp : rp + 1, mlo : mlo + 64], 1.0)
        sel = const_p.tile([128, 128], dt.float32r, tag="sel")
        nc.vector.tensor_copy(sel[:], sel_f[:])
        # 1/den staging rows at partitions 0/32/64/96 (one per head)
        recip4 = const_p.tile([128, 512], dt.float32, tag="recip4")
        nc.vector.memset(recip4[:], 1.0)
        recip4r = const_p.tile([128, 512], dt.float32r, tag="recip4r")
        nc.vector.tensor_copy(recip4r[:], recip4[:])

        # ---- persistent activations ----
        # qkt tiles: 0: q heads 0,1 | 1: q heads 2,3 | 2: k heads 0,1 | 3: k heads 2,3
        qkt = [qkt_p.tile([128, S], dt.float32r, tag=f"qkt{f}", name=f"qkt{f}") for f in range(4)]
        # vones[t]: [v h0 |1| v h1 |1| v h2 |1| v h3 |1] for token tile t
        vones = [vones_p.tile([128, 4 * 65], dt.float32r, tag=f"v{t}", name=f"v{t}") for t in range(NT)]
        # pair tiles: final normalized attn output, [head dims x 2, S]
        pairt = [pair_p.tile([128, S], dt.float32r, tag=f"pair{hp}", name=f"pair{hp}") for hp in range(2)]

        env = dict(
            x_d=x_d, wqk_sb=wqk_sb, wv_sb=wv_sb, bqk_sb=bqk_sb, ones_sb=ones_sb,
            xr_p=xr_p, qkt=qkt, vones=vones, xr_pre={0: xr0, 1: xr1},
            ps_p=ps_p, po_p=po_p, attn_p=attn_p, small_p=small_p,
            pairt=pairt, bv_sb=bv_sb, mask_sb=mask_sb, wo_r=wo_r,
            y_p=y_p, y_d=y_d, recip4=recip4, recip4r=recip4r, sel=sel,
        )
        # startup: chunk-0 qkv emitted directly
        for u in self.qkv_units(0, env):
            u()
        # filler composition per chunk: a few qkv units first (so the
        # deferred rollout's recip is ready before its bch matmul), then
        # rollout phase-2, then the rest.  oproj(1) is deferred from chunk 2
        # to chunk 3 to balance PE work against chunk 3's exp-heavy scalar
        # load.
        pending_rollout = []
        oproj_sched = {1: [0], 2: [], 3: [1, 2]}
        for c in range(NCH):
            qkv = self.qkv_units(c + 1, env) if c + 1 < NCH else []
            op = []
            for oc in oproj_sched.get(c, []):
                op += self.oproj_units(oc, env)
            if qkv:
                fillers = qkv[0:4] + pending_rollout + qkv[4:] + op
            else:
                fillers = op[0:2] + pending_rollout + op[2:]
            pending_rollout = self.attention_chunk(c, env, fillers)
        for u in pending_rollout:
            u()
        for u in self.oproj_units(NCH - 1, env, final=True):
            u()

    # ------------------------------------------------------------------
    def qkv_units(self, c, env):
        nc = self.nc
        cs = slice(512 * c, 512 * (c + 1))
        x_d, wqk_sb, wv_sb = env["x_d"], env["wqk_sb"], env["wv_sb"]
        qkt, vones = env["qkt"], env["vones"]
        bqk_sb, ones_sb = env["bqk_sb"], env["ones_sb"]
        xr_p, ps_p = env["xr_p"], env["ps_p"]

        if c in env["xr_pre"]:
            xr = env["xr_pre"][c]
        else:
            # issue the prefetch immediately (gpsimd SWDGE ring) instead of
            # weaving it, so the transfer is long done before the units run
            xr = xr_p.tile([128, KE, 512], dt.float32r, tag="xr", name="xr")
            nc.gpsimd.dma_start(xr[:], x_d[:, c])
        units = []

        def qk_unit(f):
            pq = ps_p.tile([128, 1024], dt.float32, tag="ps", name="pq")
            for ke in range(KE):
                nc.tensor.matmul(
                    pq[:, 0:512], wqk_sb[:, f, ke], xr[:, ke],
                    start=(ke == 0), stop=(ke == KE - 1),
                )
            nc.vector.tensor_scalar_add(qkt[f][:, cs], pq[:, 0:512], bqk_sb[:, f : f + 1])

        def v_unit(t4):
            t = 4 * c + t4
            pv = ps_p.tile([128, 1024], dt.float32, tag="ps", name="pv")
            for ke in range(KE):
                nc.tensor.matmul(
                    pv[:, 0:FV],
                    xr[:, ke, 128 * t4 : 128 * (t4 + 1)], wv_sb[:, ke],
                    start=(ke == 0), stop=(ke == KE - 1),
                )
            vt = vones[t]
            v3 = vt[:].rearrange("p (g d) -> p g d", d=65)
            nc.vector.tensor_copy(
                v3[:, :, 0:64],
                pv[:, 0:FV].rearrange("p (g d) -> p g d", d=64),
            )
            nc.vector.tensor_copy(v3[:, :, 64:65], ones_sb[:].to_broadcast((128, 4, 1)))

        for f in range(4):
            units.append(lambda f=f: qk_unit(f))
        for t4 in range(4):
            units.append(lambda t4=t4: v_unit(t4))
        return units

    # ------------------------------------------------------------------
    def oproj_units(self, c, env, final=False):
        nc = self.nc
        pairt, wo_r, ps_p, y_p, y_d = (
            env["pairt"], env["wo_r"], env["ps_p"], env["y_p"], env["y_d"])
        units = []
        ysbs = {}

        def unit(t4, o):
            t = 4 * c + t4
            if o == 0:
                ysbs[t4] = y_p.tile([128, E], dt.float32, tag="y", name="ysb")
            ysb = ysbs[t4]
            py = ps_p.tile([128, 1024], dt.float32, tag="ps", name="py")
            for kt in range(2):
                nc.tensor.matmul(
                    py[:, 0:512],
                    pairt[kt][:, 128 * t : 128 * (t + 1)],
                    wo_r[kt][:, 512 * o : 512 * (o + 1)],
                    start=(kt == 0), stop=(kt == 1),
                )
            nc.vector.tensor_copy(ysb[:, 512 * o : 512 * (o + 1)], py[:, 0:512])
            if o == 1:
                if final:
                    eng = (nc.sync, nc.gpsimd, nc.scalar, nc.gpsimd)[t % 4]
                else:
                    eng = (nc.sync, nc.gpsimd)[t % 2]
                eng.dma_start(y_d[128 * t : 128 * (t + 1), :], ysb[:])

        for t4 in range(4):
            for o in range(2):
                units.append(lambda t4=t4, o=o: unit(t4, o))
        return units

    # ------------------------------------------------------------------
    def attention_chunk(self, c, env, fillers):
        """Attention for both head pairs of chunk c, weaving filler units
        (next-chunk qkv / prev-chunk out-proj) into the PE stream."""
        nc = self.nc
        qkt, vones = env["qkt"], env["vones"]
        ps_p, po_p, attn_p, small_p = (
            env["ps_p"], env["po_p"], env["attn_p"], env["small_p"])
        pairt, bv_sb, mask_sb = env["pairt"], env["bv_sb"], env["mask_sb"]
        nj = 4 * c + 4
        # po[2*hp + h_idx]: [65, 512] accumulator per head
        po = [po_p.tile([65, 512], dt.float32, tag=f"po{i}", name=f"po{i}")
              for i in range(4)]

        nreserve = min(2, len(fillers))
        nfill = len(fillers) - nreserve
        iters = 2 * nj
        emitted = 0

        def emit_pv(hp, j, off, at):
            for h_idx in range(2):
                slot = 2 * hp + h_idx
                nc.tensor.matmul(
                    po[slot][:, off:512],
                    vones[j][:, 65 * slot : 65 * slot + 65],
                    at[:, 512 * h_idx + off : 512 * (h_idx + 1)],
                    start=(j == 0), stop=(j == nj - 1),
                    skip_group_check=True,
                )

        it = 0
        pending = []
        for j in range(nj):
            for hp in range(2):
                ps = ps_p.tile([128, 1024], dt.float32, tag="ps", name="ps")
                at = attn_p.tile([128, 1024], dt.float32r, tag="attn", name="at")
                m = j - 4 * c
                off = 128 * m if 1 <= m <= 3 else 0
                for h_idx in range(2):
                    r0 = 64 * h_idx
                    nc.tensor.matmul(
                        ps[:, 512 * h_idx + off : 512 * (h_idx + 1)],
                        qkt[2 + hp][r0 : r0 + 64, 128 * j : 128 * (j + 1)],
                        qkt[hp][r0 : r0 + 64, 512 * c + off : 512 * (c + 1)],
                        start=True, stop=True,
                    )
                if m >= 0:
                    lo = 128 * m
                    nc.vector.tensor_add(
                        ps[:].rearrange("p (g n) -> p g n", g=2)[:, :, lo : lo + 128],
                        ps[:].rearrange("p (g n) -> p g n", g=2)[:, :, lo : lo + 128],
                        mask_sb[:].rearrange("p (o n) -> p o n", o=1)
                        .to_broadcast((128, 2, 128)))
                if off == 0:
                    runs = [(0, 1024)]
                else:
                    runs = [(off, 512), (512 + off, 1024)]
                for lo, hi in runs:
                    nc.scalar.activation(
                        at[:, lo:hi], ps[:, lo:hi], mybir.ActivationFunctionType.Exp)
                pending.append((hp, j, off, at))
                if len(pending) > PDEPTH:
                    emit_pv(*pending.pop(0))
                it += 1
                while emitted < nfill and emitted * iters < it * nfill:
                    fillers[emitted]()
                    emitted += 1
        for p in pending:
            emit_pv(*p)
        while emitted < nfill:
            fillers[emitted]()
            emitted += 1
        # reserved fillers keep the PE stream dense across the chunk
        # boundary (HAM re-throttles if PE density dips for ~3.4us)
        for u in fillers[nfill:]:
            u()
        # ---- rollout: drain po to SBUF fast (frees PSUM for next chunk),
        # then normalize off SBUF while the next chunk proceeds ----
        recip4 = env["recip4"]
        for i in range(4):
            nc.vector.tensor_copy(recip4[32 * i : 32 * i + 1, :], po[i][64:65, :])
        po_sb = [
            small_p.tile([128, 512], dt.float32, tag=f"posb{hp}", name=f"posb{hp}")
            for hp in range(2)
        ]
        for hp in range(2):
            nc.vector.tensor_copy(po_sb[hp][0:64, :], po[2 * hp][0:64, :])
            nc.vector.tensor_copy(po_sb[hp][64:128, :], po[2 * hp + 1][0:64, :])
        # phase 2 (normalize off SBUF) is returned as filler units for the
        # next chunk so the PE stream never pauses at the chunk boundary
        recip4r = env["recip4r"]
        sel = env["sel"]
        # cheap approx reciprocal (full reciprocal has ~3.5us fixed cost);
        # vector-only, so it runs during the next chunk's first iterations
        nc.vector.reciprocal_approx_fast(recip4[:], recip4[:])
        nc.vector.tensor_copy(recip4r[:], recip4[:])

        bch_ps_box = []

        def u_bch():
            # broadcast 1/den across partitions with one selector matmul per
            # head pair (avoids slow 64-descriptor broadcast DMAs)
            bch_ps = ps_p.tile([128, 1024], dt.float32, tag="ps", name="bch")
            bch_ps_box.append(bch_ps)
            for hp in range(2):
                nc.tensor.matmul(
                    bch_ps[:, 512 * hp : 512 * (hp + 1)],
                    sel[64 * hp : 64 * hp + 33, :],
                    recip4r[64 * hp : 64 * hp + 33, :],
                    start=True, stop=True, skip_group_check=True,
                )

        def u_norm(hp):
            bch_ps = bch_ps_box[0]
            tmp = small_p.tile([128, 512], dt.float32, tag=f"tmp{hp}", name=f"tmp{hp}")
            nc.vector.tensor_mul(
                tmp[:], po_sb[hp][:], bch_ps[:, 512 * hp : 512 * (hp + 1)])
            nc.vector.tensor_scalar_add(
                pairt[hp][:, 512 * c : 512 * (c + 1)], tmp[:], bv_sb[:, hp : hp + 1]
            )

        return [u_bch] + [lambda hp=hp: u_norm(hp) for hp in range(2)]

# ----------------------------------------------------------------------
_PROGRAM = None


def _get_program():
    global _PROGRAM
    if _PROGRAM is None:
        _PROGRAM = _build_program()
    return _PROGRAM


def _make_in_maps(inputs, W_in, b_in, W_out, b_out):
    in_maps = []
    scale = 1.0 / np.sqrt(np.float32(HD))
    kr = np.arange(128)[:, None]
    qc = np.arange(128)[None, :]
    trimask = np.where(qc >= kr, 0.0, -1e30).astype(np.float32)
    for core in range(NC):
        b, g = divmod(core, 4)
        r = slice(256 * g, 256 * (g + 1))
        wq = W_in[0:E][r] * scale
        wk = W_in[E : 2 * E][r]
        wv = W_in[2 * E : 3 * E][r]
        xT = inputs[b].T.astype(np.float32)               # [E, S]
        wqkT = np.concatenate([wq, wk], axis=0).T         # [E, FQK]
        wvT = wv.T                                        # [E, FV]
        wo = W_out[:, r].T                                # [FV, E]
        # shuffle so each partition's DMA payload is contiguous
        x_h = np.ascontiguousarray(
            xT.reshape(KE, 128, NCH, 512).transpose(1, 2, 0, 3))
        wqk_h = np.ascontiguousarray(
            wqkT.reshape(KE, 128, 4, 128).transpose(1, 2, 0, 3).astype(np.float32))
        wv_h = np.ascontiguousarray(
            wvT.reshape(KE, 128, FV).transpose(1, 0, 2).astype(np.float32))
        wo_h = np.ascontiguousarray(
            wo.reshape(2, 128, E).transpose(1, 0, 2).astype(np.float32))
        bqk = np.concatenate([b_in[0:E][r] * scale, b_in[E : 2 * E][r]])
        bqk_h = np.ascontiguousarray(bqk.reshape(4, 128).T.astype(np.float32))
        bv_h = np.ascontiguousarray(
            b_in[2 * E : 3 * E][r].reshape(2, 128).T.astype(np.float32))
        in_maps.append(
            {
                "x": x_h,
                "wqk": wqk_h,
                "wv": wv_h,
                "wo": wo_h,
                "bqk": bqk_h,
                "bv": bv_h,
                "trimask": trimask,
            }
        )
    return in_maps


def run_spmd(inputs, W_in, b_in, W_out, b_out, trace=False, **kw):
    nc = _get_program()
    in_maps = _make_in_maps(inputs, W_in, b_in, W_out, b_out)
    bkr = run_bass_kernel_spmd(nc, in_maps, list(range(NC)), trace=trace, **kw)
    parts = [bkr.results[i]["y"] for i in range(NC)]
    out = np.stack(
        [
            parts[0] + parts[1] + parts[2] + parts[3],
            parts[4] + parts[5] + parts[6] + parts[7],
        ]
    )
    out = out + b_out[None, None, :]
    return out.astype(np.float32), bkr


def kernel(inputs, W_in, b_in, W_out, b_out):
    out, _ = run_spmd(
        np.asarray(inputs, dtype=np.float32),
        np.asarray(W_in, dtype=np.float32),
        np.asarray(b_in, dtype=np.float32),
        np.asarray(W_out, dtype=np.float32),
        np.asarray(b_out, dtype=np.float32),
    )
    return out


if __name__ == "__main__":
    import reference
    ins = {k: np.asarray(v) for k, v in reference.setup_inputs().items()}
    exp = np.asarray(reference.reference(**ins))
    got = kernel(**ins)
    rel = np.abs(got - exp).max() / np.abs(exp).max()
    print("rel", rel)
